# revision 29
# baseline (speedup 1.0000x reference)
"""MoE layer (B=8192, D=1024, E=8, top-2, H=2048) on 8 TRN2 NeuronCores.

Strategy (expert-parallel with two-segment load balancing):
  - Host: gate logits = x @ Wg (fp32), exact top-2 (jax tie-break semantics),
    softmax over the 2 picks. The 16384 (token, expert) pairs are grouped by
    expert and each expert's list is cut into pieces assigned to 16 fixed-size
    segments (8 of size S1, 8 of size S2, one of each per core) so every core
    carries ~the mean load instead of the max expert's load.
  - Device (SPMD): each core runs two segments; segment rows use that
    segment's expert weights: y = relu(x @ W1[e] + b1[e]) @ W2[e] in bf16
    with fp32 PSUM accumulation. Weights resident in SBUF, tokens processed
    in column chunks of <=512.
  - Host: weighted combine out[tok] = sum_k gate * (y + b2[e]).

The expert MLPs are the only O(B*D*H) work; gating/combine are O(B*D).
"""

import os

import numpy as np
import ml_dtypes

B, D, E, TOP_K = 8192, 1024, 8, 2
H = 2 * D
P = 128
CHUNK = 512

KD = D // P  # 8 contraction tiles for mm1 (over D)
MH = H // P  # 16 output tiles for mm1 / contraction tiles for mm2 (over H)
MD = D // P  # 8 output tiles for mm2 (over D)

_BF16 = np.dtype(ml_dtypes.bfloat16)

LAST_RESULTS = None  # BassKernelResults of the most recent run (for test harness)


def _chunk_sizes(n):
    """Split n columns into matmul chunks <=512, avoiding tiny tails."""
    chunks = [CHUNK] * (n // CHUNK)
    tail = n % CHUNK
    if tail >= 128 or not chunks:
        if tail:
            chunks.append(tail)
    elif tail:
        last = chunks.pop() + tail
        chunks += [last - last // 2, last // 2]
    return chunks


def _plan_segments(counts):
    """Pick segment sizes (S1 >= S2) and cut experts into 8 S1-pieces and
    8 S2-pieces (one of each per core). Returns (S1, S2, pieces) where
    pieces[core] = [(expert, tok_start, fill_len), (expert, tok_start, fill_len)]
    for the S1 and S2 segment respectively."""
    order = np.argsort(-counts, kind="stable")
    best = None
    for k in range(0, E // 2 + 1):
        big = order[:k]
        small = order[E - k :] if k else order[:0]
        mid = order[k : E - k]
        S1 = max((int(-(-counts[e] // 2)) for e in big), default=0)
        S2 = max((int(-(-counts[e] // 2)) for e in small), default=0)
        if len(mid):
            if S1 == 0:  # k == 0: all experts are (S1, S2)
                S1 = int(-(-counts[mid].max() // 2))
            S2 = max(S2, int(counts[mid].max()) - S1)
        C = S1 + S2
        if best is None or C < best[0]:
            best = (C, k, S1, S2)
    _, k, S1, S2 = best
    S1 = int(-(-S1 // 8) * 8)  # align for DMA friendliness
    S2 = int(-(-S2 // 8) * 8)

    big = order[:k]
    small = order[E - k :] if k else order[:0]
    mid = order[k : E - k]
    s1_pieces = []  # (expert, tok_start, len)
    s2_pieces = []
    for e in big:
        n = int(counts[e])
        h1 = -(-n // 2)
        s1_pieces += [(int(e), 0, h1), (int(e), h1, n - h1)]
    for e in small:
        n = int(counts[e])
        h1 = -(-n // 2)
        s2_pieces += [(int(e), 0, h1), (int(e), h1, n - h1)]
    for e in mid:
        n = int(counts[e])
        a = min(n, S1)
        s1_pieces.append((int(e), 0, a))
        s2_pieces.append((int(e), a, n - a))
    assert len(s1_pieces) == E and len(s2_pieces) == E
    assert all(ln <= S1 for _, _, ln in s1_pieces)
    assert all(ln <= S2 for _, _, ln in s2_pieces)
    pieces = [[s1_pieces[i], s2_pieces[i]] for i in range(E)]
    return S1, S2, pieces


def _build_program(S1, S2):
    import concourse.bacc as bacc
    import concourse.mybir as mybir
    import concourse.tile as tile
    from concourse.bass import ts

    C = S1 + S2
    nc = bacc.Bacc(
        "TRN2", target_bir_lowering=False, debug=False, enable_asserts=False
    )
    bf16 = mybir.dt.bfloat16
    f32 = mybir.dt.float32

    xt_d = nc.dram_tensor("xt", (D, C), bf16, kind="ExternalInput").ap()
    w1_ds, w2_ds, b1_ds = [], [], []
    for s in ("a", "b"):
        w1_ds.append(nc.dram_tensor(f"w1{s}", (D, H), bf16, kind="ExternalInput").ap())
        w2_ds.append(nc.dram_tensor(f"w2{s}", (H, D), bf16, kind="ExternalInput").ap())
        b1_ds.append(
            nc.dram_tensor(f"b1{s}r", (P, MH), f32, kind="ExternalInput").ap()
        )
    yt_d = nc.dram_tensor("yt", (D, C), f32, kind="ExternalOutput").ap()

    with tile.TileContext(nc) as tc:
        with (
            tc.tile_pool(name="weights", bufs=1) as wpool,
            tc.tile_pool(name="xin", bufs=1) as xpool,
            tc.tile_pool(name="hbuf", bufs=1) as hpool,
            tc.tile_pool(name="ystage", bufs=4) as ypool,
            tc.tile_pool(name="ps", bufs=8, space="PSUM") as pspool,
        ):
            xt_sb = xpool.tile([P, KD, C], bf16)
            xt_r = xt_d.rearrange("(ko p) c -> p ko c", p=P)
            w1_sbs, w2_sbs, b1_sbs = [], [], []
            w1_rs, w2_rs = [], []
            for s in range(2):
                w1_sbs.append(wpool.tile([P, KD, H], bf16, name=f"w1sb{s}"))
                w2_sbs.append(wpool.tile([P, MH, D], bf16, name=f"w2sb{s}"))
                b1_sbs.append(wpool.tile([P, MH], f32, name=f"b1sb{s}"))
                w1_rs.append(w1_ds[s].rearrange("(ko p) h -> p ko h", p=P))
                w2_rs.append(w2_ds[s].rearrange("(ko p) d -> p ko d", p=P))

            # DMA issue order == need order, alternating pieces of each class
            # between sync (8 HWDGE queues) and gpsimd (8 SWDGE queues) so the
            # early phase gets ~16-queue aggregate bandwidth. The first ~40us
            # of compute consumes w1a+xt(c0)+w2a = 9MB (~250GB/s demand),
            # above the 8-queue ceiling.
            def dma(i, dst, src):
                nc.sync.dma_start(dst, src)

            for i, k in enumerate(range(0, KD, 2)):  # w1a m-group 0: 4 x 64KB
                dma(i, w1_sbs[0][:, k : k + 2, 0:128], w1_rs[0][:, k : k + 2, 0:128])
            for k in range(KD):  # xt chunk 0: 8 x 128KB
                dma(k, xt_sb[:, k, 0:CHUNK], xt_r[:, k, 0:CHUNK])
            for i, k in enumerate(range(0, KD, 2)):  # w1a m-groups 1..3
                dma(
                    i,
                    w1_sbs[0][:, k : k + 2, 128:512],
                    w1_rs[0][:, k : k + 2, 128:512],
                )
            for k in range(KD):  # w1a m-groups 4..15: 8 x 384KB
                dma(k, w1_sbs[0][:, k, 512:H], w1_rs[0][:, k, 512:H])
            dma(0, b1_sbs[0], b1_ds[0])
            dma(1, b1_sbs[1], b1_ds[1])
            for i, k in enumerate(range(0, MH, 2)):  # w2a: 8 x 512KB (~40us in)
                dma(i, w2_sbs[0][:, k : k + 2], w2_rs[0][:, k : k + 2])
            for k in range(KD):  # xt remaining cols (chunk 1 needed ~65us in)
                dma(k, xt_sb[:, k, CHUNK:C], xt_r[:, k, CHUNK:C])
            for k in range(KD):  # segment B weights: needed ~120us in
                dma(k, w1_sbs[1][:, k], w1_rs[1][:, k])
            for i, k in enumerate(range(0, MH, 2)):
                dma(i, w2_sbs[1][:, k : k + 2], w2_rs[1][:, k : k + 2])

            # Chunk schedule: mm2 lags mm1 by one chunk. This keeps w2a off
            # the first ~65us of the DMA critical path and the PE never waits
            # on the ACT h-tiles of the chunk it just produced.
            chunk_list = []  # (seg, off, tw)
            off = 0
            for seg, seg_len in ((0, S1), (1, S2)):
                for tw in _chunk_sizes(seg_len):
                    chunk_list.append((seg, off, tw))
                    off += tw

            def mm2_phase(seg, off, tw, h_sb, k2_outer):
                # k2-outer: all 8 output banks accumulate together so each
                # w2[k2] slice is consumed as it lands (spreads the 4MB w2
                # demand over ~27us instead of needing it all upfront). The
                # last chunk uses m2-outer instead so its copies/output DMAs
                # overlap its own matmul stream rather than trailing it.
                w2_sb = w2_sbs[seg]
                if k2_outer:
                    pys = [
                        pspool.tile([P, CHUNK], f32, tag="ps", name=f"py{m2}")
                        for m2 in range(MD)
                    ]
                    for k2 in range(MH):
                        for m2 in range(MD):
                            nc.tensor.matmul(
                                pys[m2][:, :tw],
                                w2_sb[:, k2, ts(m2, P)],
                                h_sb[:, k2, :tw],
                                start=(k2 == 0),
                                stop=(k2 == MH - 1),
                            )
                    for m2 in range(MD):
                        y_sb = ypool.tile([P, CHUNK], f32, tag="y")
                        nc.vector.tensor_copy(y_sb[:, :tw], pys[m2][:, :tw])
                        nc.sync.dma_start(
                            yt_d[ts(m2, P), off : off + tw], y_sb[:, :tw]
                        )
                else:
                    for m2 in range(MD):
                        py = pspool.tile([P, CHUNK], f32, tag="ps", name="py")
                        for k2 in range(MH):
                            nc.tensor.matmul(
                                py[:, :tw],
                                w2_sb[:, k2, ts(m2, P)],
                                h_sb[:, k2, :tw],
                                start=(k2 == 0),
                                stop=(k2 == MH - 1),
                            )
                        y_sb = ypool.tile([P, CHUNK], f32, tag="y")
                        nc.vector.tensor_copy(y_sb[:, :tw], py[:, :tw])
                        nc.sync.dma_start(
                            yt_d[ts(m2, P), off : off + tw], y_sb[:, :tw]
                        )

            # PE warmup: ~45 junk matmuls on a memset tile run while the first
            # weight/activation DMAs land, so the HAM clock gate is already at
            # 8/8 when real matmuls start.
            warm_sb = xpool.tile([P, P], bf16, name="warm")
            nc.vector.memset(warm_sb, 0.0)
            warm_ps = pspool.tile([P, P], f32, tag="ps", name="warm_ps")
            for _ in range(90):
                nc.tensor.matmul(warm_ps, warm_sb, warm_sb, start=True, stop=True)

            for ci, (seg, off, tw) in enumerate(chunk_list):
                w1_sb, b1_sb = w1_sbs[seg], b1_sbs[seg]
                h_sb = hpool.tile([P, MH, CHUNK], bf16, tag="h")
                for m in range(MH):
                    ph = pspool.tile([P, CHUNK], f32, tag="ps", name="ph")
                    for k in range(KD):
                        nc.tensor.matmul(
                            ph[:, :tw],
                            w1_sb[:, k, ts(m, P)],
                            xt_sb[:, k, off : off + tw],
                            start=(k == 0),
                            stop=(k == KD - 1),
                        )
                    nc.scalar.activation(
                        h_sb[:, m, :tw],
                        ph[:, :tw],
                        mybir.ActivationFunctionType.Relu,
                        bias=b1_sb[:, m : m + 1],
                    )
                    if ci == 0 and m < 10:
                        # Fill DMA-ramp bubbles with dependency-free matmuls
                        # so the HAM clock gate stays at 8/8 while chunk-0
                        # weights stream in.
                        for _ in range(6):
                            nc.tensor.matmul(
                                warm_ps, warm_sb, warm_sb, start=True, stop=True
                            )
                mm2_phase(seg, off, tw, h_sb, k2_outer=(ci < len(chunk_list) - 1))
    nc.finalize()
    return nc


def _route(x, Wg):
    """Exact reference gating on host: top-2 of clean fp32 logits (jax
    tie-break: lower index first), softmax over the two picks."""
    logits = x @ Wg  # [B, E] fp32
    order = np.argsort(-logits, axis=1, kind="stable")[:, :TOP_K]  # [B, 2]
    top_vals = np.take_along_axis(logits, order, axis=1)
    ex = np.exp(top_vals - top_vals[:, :1])  # top_vals sorted desc -> max first
    gates = (ex / ex.sum(axis=1, keepdims=True)).astype(np.float32)  # [B, 2]
    return order, gates


def kernel(x, Wg, W1, b1, W2, b2):
    x = np.ascontiguousarray(np.asarray(x, dtype=np.float32))
    Wg = np.asarray(Wg, dtype=np.float32)
    W1 = np.asarray(W1, dtype=np.float32)
    b1 = np.asarray(b1, dtype=np.float32)
    W2 = np.asarray(W2, dtype=np.float32)
    b2 = np.asarray(b2, dtype=np.float32)

    order, gates = _route(x, Wg)

    # Dispatch: flatten (token, k) pairs, bucket by expert (stable => slot
    # order within an expert follows token order). Pair p belongs to token p//2.
    expert_flat = order.reshape(-1)  # [2B]
    gate_flat = gates.reshape(-1)  # [2B]
    perm = np.argsort(expert_flat, kind="stable")  # pairs grouped by expert
    counts = np.bincount(expert_flat, minlength=E)
    offs = np.concatenate(([0], np.cumsum(counts)))[:E]

    S1, S2, pieces = _plan_segments(counts)
    C = S1 + S2
    assert C <= 4864, f"unexpectedly imbalanced routing: {counts}"

    # Per-pair placement (core, column) for the combine step, and per-core
    # token lists for the dispatch.
    core_of_pair = np.empty(2 * B, dtype=np.int64)
    col_of_pair = np.empty(2 * B, dtype=np.int64)
    xT = np.ascontiguousarray(x.T)  # [D, B]
    bf16_w1 = [None] * E
    bf16_w2 = [None] * E
    in_maps = []
    for core in range(E):
        xg = np.zeros((D, C), dtype=_BF16)
        in_map = {"xt": xg}
        for seg, (e, tok_start, ln) in enumerate(pieces[core]):
            seg_off = 0 if seg == 0 else S1
            if ln:
                pair_idx = perm[offs[e] + tok_start : offs[e] + tok_start + ln]
                toks = pair_idx // 2
                xg[:, seg_off : seg_off + ln] = xT[:, toks].astype(_BF16)
                core_of_pair[pair_idx] = core
                col_of_pair[pair_idx] = seg_off + np.arange(ln)
            if bf16_w1[e] is None:
                bf16_w1[e] = W1[e].astype(_BF16)
                bf16_w2[e] = W2[e].astype(_BF16)
            s = "ab"[seg]
            in_map[f"w1{s}"] = bf16_w1[e]
            in_map[f"w2{s}"] = bf16_w2[e]
            in_map[f"b1{s}r"] = np.ascontiguousarray(b1[e].reshape(MH, P).T)
        in_maps.append(in_map)

    nc = _build_program(S1, S2)

    from concourse.bass_utils import run_bass_kernel_spmd

    trace = os.environ.get("MOE_TRACE") == "1"
    kwargs = {}
    if trace:
        kwargs = dict(trace=True, trace_cores=list(range(E)))
    res = run_bass_kernel_spmd(nc, in_maps, core_ids=list(range(E)), **kwargs)
    global LAST_RESULTS
    LAST_RESULTS = res

    Y = np.stack([r["yt"] for r in res.results])  # [E, D, C] f32

    # Combine: pair p contributes gate_p * (y[:, col_p] + b2[e_p]) to token
    # p//2. Pairs of token b sit at flat positions 2b, 2b+1.
    cols = Y[core_of_pair, :, col_of_pair]  # [2B, D]
    weighted = (cols + b2[expert_flat]) * gate_flat[:, None]
    out = weighted[0::2] + weighted[1::2]
    return np.ascontiguousarray(out, dtype=np.float32)


# revision 32
# speedup vs baseline: 1.0012x; 1.0012x over previous
"""MoE layer (B=8192, D=1024, E=8, top-2, H=2048) on 8 TRN2 NeuronCores.

Strategy (expert-parallel with two-segment load balancing):
  - Host: gate logits = x @ Wg (fp32), exact top-2 (jax tie-break semantics),
    softmax over the 2 picks. The 16384 (token, expert) pairs are grouped by
    expert and each expert's list is cut into pieces assigned to 16 fixed-size
    segments (8 of size S1, 8 of size S2, one of each per core) so every core
    carries ~the mean load instead of the max expert's load.
  - Device (SPMD): each core runs two segments; segment rows use that
    segment's expert weights: y = relu(x @ W1[e] + b1[e]) @ W2[e] in bf16
    with fp32 PSUM accumulation. Weights resident in SBUF, tokens processed
    in column chunks of <=512.
  - Host: weighted combine out[tok] = sum_k gate * (y + b2[e]).

The expert MLPs are the only O(B*D*H) work; gating/combine are O(B*D).
"""

import os

import numpy as np
import ml_dtypes

B, D, E, TOP_K = 8192, 1024, 8, 2
H = 2 * D
P = 128
CHUNK = 512

KD = D // P  # 8 contraction tiles for mm1 (over D)
MH = H // P  # 16 output tiles for mm1 / contraction tiles for mm2 (over H)
MD = D // P  # 8 output tiles for mm2 (over D)

_BF16 = np.dtype(ml_dtypes.bfloat16)

LAST_RESULTS = None  # BassKernelResults of the most recent run (for test harness)


def _chunk_sizes(n):
    """Split n columns into matmul chunks <=512, avoiding tiny tails."""
    chunks = [CHUNK] * (n // CHUNK)
    tail = n % CHUNK
    if tail >= 128 or not chunks:
        if tail:
            chunks.append(tail)
    elif tail:
        last = chunks.pop() + tail
        chunks += [last - last // 2, last // 2]
    return chunks


def _plan_segments(counts):
    """Pick segment sizes (S1 >= S2) and cut experts into 8 S1-pieces and
    8 S2-pieces (one of each per core). Returns (S1, S2, pieces) where
    pieces[core] = [(expert, tok_start, fill_len), (expert, tok_start, fill_len)]
    for the S1 and S2 segment respectively."""
    order = np.argsort(-counts, kind="stable")
    best = None
    for k in range(0, E // 2 + 1):
        big = order[:k]
        small = order[E - k :] if k else order[:0]
        mid = order[k : E - k]
        S1 = max((int(-(-counts[e] // 2)) for e in big), default=0)
        S2 = max((int(-(-counts[e] // 2)) for e in small), default=0)
        if len(mid):
            if S1 == 0:  # k == 0: all experts are (S1, S2)
                S1 = int(-(-counts[mid].max() // 2))
            S2 = max(S2, int(counts[mid].max()) - S1)
        C = S1 + S2
        if best is None or C < best[0]:
            best = (C, k, S1, S2)
    _, k, S1, S2 = best
    S1 = int(-(-S1 // 8) * 8)  # align for DMA friendliness
    S2 = int(-(-S2 // 8) * 8)

    big = order[:k]
    small = order[E - k :] if k else order[:0]
    mid = order[k : E - k]
    s1_pieces = []  # (expert, tok_start, len)
    s2_pieces = []
    for e in big:
        n = int(counts[e])
        h1 = -(-n // 2)
        s1_pieces += [(int(e), 0, h1), (int(e), h1, n - h1)]
    for e in small:
        n = int(counts[e])
        h1 = -(-n // 2)
        s2_pieces += [(int(e), 0, h1), (int(e), h1, n - h1)]
    for e in mid:
        n = int(counts[e])
        a = min(n, S1)
        s1_pieces.append((int(e), 0, a))
        s2_pieces.append((int(e), a, n - a))
    assert len(s1_pieces) == E and len(s2_pieces) == E
    assert all(ln <= S1 for _, _, ln in s1_pieces)
    assert all(ln <= S2 for _, _, ln in s2_pieces)
    pieces = [[s1_pieces[i], s2_pieces[i]] for i in range(E)]
    return S1, S2, pieces


def _build_program(S1, S2):
    import concourse.bacc as bacc
    import concourse.mybir as mybir
    import concourse.tile as tile
    from concourse.bass import ts

    C = S1 + S2
    nc = bacc.Bacc("TRN2", target_bir_lowering=False, debug=False)
    bf16 = mybir.dt.bfloat16
    f32 = mybir.dt.float32

    xt_d = nc.dram_tensor("xt", (D, C), bf16, kind="ExternalInput").ap()
    w1_ds, w2_ds, b1_ds = [], [], []
    for s in ("a", "b"):
        w1_ds.append(nc.dram_tensor(f"w1{s}", (D, H), bf16, kind="ExternalInput").ap())
        w2_ds.append(nc.dram_tensor(f"w2{s}", (H, D), bf16, kind="ExternalInput").ap())
        b1_ds.append(
            nc.dram_tensor(f"b1{s}r", (P, MH), f32, kind="ExternalInput").ap()
        )
    yt_d = nc.dram_tensor("yt", (D, C), f32, kind="ExternalOutput").ap()

    with tile.TileContext(nc) as tc:
        with (
            tc.tile_pool(name="weights", bufs=1) as wpool,
            tc.tile_pool(name="xin", bufs=1) as xpool,
            tc.tile_pool(name="hbuf", bufs=1) as hpool,
            tc.tile_pool(name="ystage", bufs=3) as ypool,
            tc.tile_pool(name="ps", bufs=8, space="PSUM") as pspool,
        ):
            xt_sb = xpool.tile([P, KD, C], bf16)
            xt_r = xt_d.rearrange("(ko p) c -> p ko c", p=P)
            w1_sbs, w2_sbs, b1_sbs = [], [], []
            w1_rs, w2_rs = [], []
            for s in range(2):
                w1_sbs.append(wpool.tile([P, KD, H], bf16, name=f"w1sb{s}"))
                w2_sbs.append(wpool.tile([P, MH, D], bf16, name=f"w2sb{s}"))
                b1_sbs.append(wpool.tile([P, MH], f32, name=f"b1sb{s}"))
                w1_rs.append(w1_ds[s].rearrange("(ko p) h -> p ko h", p=P))
                w2_rs.append(w2_ds[s].rearrange("(ko p) d -> p ko d", p=P))

            # DMA issue order == need order, alternating pieces of each class
            # between sync (8 HWDGE queues) and gpsimd (8 SWDGE queues) so the
            # early phase gets ~16-queue aggregate bandwidth. The first ~40us
            # of compute consumes w1a+xt(c0)+w2a = 9MB (~250GB/s demand),
            # above the 8-queue ceiling.
            def dma(i, dst, src):
                nc.sync.dma_start(dst, src)

            for i, k in enumerate(range(0, KD, 2)):  # w1a m-group 0: 4 x 64KB
                dma(i, w1_sbs[0][:, k : k + 2, 0:128], w1_rs[0][:, k : k + 2, 0:128])
            for k in range(KD):  # xt chunk 0: 8 x 128KB
                dma(k, xt_sb[:, k, 0:CHUNK], xt_r[:, k, 0:CHUNK])
            for i, k in enumerate(range(0, KD, 2)):  # w1a m-groups 1..3
                dma(
                    i,
                    w1_sbs[0][:, k : k + 2, 128:512],
                    w1_rs[0][:, k : k + 2, 128:512],
                )
            for k in range(KD):  # w1a m-groups 4..15: 8 x 384KB
                dma(k, w1_sbs[0][:, k, 512:H], w1_rs[0][:, k, 512:H])
            dma(0, b1_sbs[0], b1_ds[0])
            dma(1, b1_sbs[1], b1_ds[1])
            for i, k in enumerate(range(0, MH, 2)):  # w2a: 8 x 512KB (~40us in)
                dma(i, w2_sbs[0][:, k : k + 2], w2_rs[0][:, k : k + 2])
            for k in range(KD):  # xt remaining cols (chunk 1 needed ~65us in)
                dma(k, xt_sb[:, k, CHUNK:C], xt_r[:, k, CHUNK:C])
            for k in range(KD):  # segment B weights: needed ~120us in
                dma(k, w1_sbs[1][:, k], w1_rs[1][:, k])
            for i, k in enumerate(range(0, MH, 2)):
                dma(i, w2_sbs[1][:, k : k + 2], w2_rs[1][:, k : k + 2])

            # Chunk schedule: mm2 lags mm1 by one chunk. This keeps w2a off
            # the first ~65us of the DMA critical path and the PE never waits
            # on the ACT h-tiles of the chunk it just produced.
            chunk_list = []  # (seg, off, tw)
            off = 0
            for seg, seg_len in ((0, S1), (1, S2)):
                for tw in _chunk_sizes(seg_len):
                    chunk_list.append((seg, off, tw))
                    off += tw

            def mm2_phase(seg, off, tw, h_sb, k2_outer):
                # k2-outer: all 8 output banks accumulate together so each
                # w2[k2] slice is consumed as it lands (spreads the 4MB w2
                # demand over ~27us instead of needing it all upfront). The
                # last chunk uses m2-outer instead so its copies/output DMAs
                # overlap its own matmul stream rather than trailing it.
                w2_sb = w2_sbs[seg]
                if k2_outer:
                    pys = [
                        pspool.tile([P, CHUNK], f32, tag="ps", name=f"py{m2}")
                        for m2 in range(MD)
                    ]
                    for k2 in range(MH):
                        for m2 in range(MD):
                            nc.tensor.matmul(
                                pys[m2][:, :tw],
                                w2_sb[:, k2, ts(m2, P)],
                                h_sb[:, k2, :tw],
                                start=(k2 == 0),
                                stop=(k2 == MH - 1),
                            )
                    for m2 in range(MD):
                        y_sb = ypool.tile([P, CHUNK], f32, tag="y")
                        nc.vector.tensor_copy(y_sb[:, :tw], pys[m2][:, :tw])
                        nc.sync.dma_start(
                            yt_d[ts(m2, P), off : off + tw], y_sb[:, :tw]
                        )
                else:
                    for m2 in range(MD):
                        py = pspool.tile([P, CHUNK], f32, tag="ps", name="py")
                        for k2 in range(MH):
                            nc.tensor.matmul(
                                py[:, :tw],
                                w2_sb[:, k2, ts(m2, P)],
                                h_sb[:, k2, :tw],
                                start=(k2 == 0),
                                stop=(k2 == MH - 1),
                            )
                        y_sb = ypool.tile([P, CHUNK], f32, tag="y")
                        nc.vector.tensor_copy(y_sb[:, :tw], py[:, :tw])
                        nc.sync.dma_start(
                            yt_d[ts(m2, P), off : off + tw], y_sb[:, :tw]
                        )

            # PE warmup: ~45 junk matmuls on a memset tile run while the first
            # weight/activation DMAs land, so the HAM clock gate is already at
            # 8/8 when real matmuls start.
            warm_sb = xpool.tile([P, P], bf16, name="warm")
            nc.vector.memset(warm_sb, 0.0)
            warm_ps = pspool.tile([P, P], f32, tag="ps", name="warm_ps")
            for _ in range(45):
                nc.tensor.matmul(warm_ps, warm_sb, warm_sb, start=True, stop=True)

            for ci, (seg, off, tw) in enumerate(chunk_list):
                w1_sb, b1_sb = w1_sbs[seg], b1_sbs[seg]
                h_sb = hpool.tile([P, MH, CHUNK], bf16, tag="h")
                for m in range(MH):
                    ph = pspool.tile([P, CHUNK], f32, tag="ps", name="ph")
                    for k in range(KD):
                        nc.tensor.matmul(
                            ph[:, :tw],
                            w1_sb[:, k, ts(m, P)],
                            xt_sb[:, k, off : off + tw],
                            start=(k == 0),
                            stop=(k == KD - 1),
                        )
                    nc.scalar.activation(
                        h_sb[:, m, :tw],
                        ph[:, :tw],
                        mybir.ActivationFunctionType.Relu,
                        bias=b1_sb[:, m : m + 1],
                    )
                    if ci == 0 and m < 10:
                        # Fill DMA-ramp bubbles with dependency-free matmuls
                        # so the HAM clock gate stays at 8/8 while chunk-0
                        # weights stream in.
                        for _ in range(6):
                            nc.tensor.matmul(
                                warm_ps, warm_sb, warm_sb, start=True, stop=True
                            )
                mm2_phase(seg, off, tw, h_sb, k2_outer=(ci < len(chunk_list) - 1))
    nc.finalize()
    return nc


def _route(x, Wg):
    """Exact reference gating on host: top-2 of clean fp32 logits (jax
    tie-break: lower index first), softmax over the two picks."""
    logits = x @ Wg  # [B, E] fp32
    order = np.argsort(-logits, axis=1, kind="stable")[:, :TOP_K]  # [B, 2]
    top_vals = np.take_along_axis(logits, order, axis=1)
    ex = np.exp(top_vals - top_vals[:, :1])  # top_vals sorted desc -> max first
    gates = (ex / ex.sum(axis=1, keepdims=True)).astype(np.float32)  # [B, 2]
    return order, gates


def kernel(x, Wg, W1, b1, W2, b2):
    x = np.ascontiguousarray(np.asarray(x, dtype=np.float32))
    Wg = np.asarray(Wg, dtype=np.float32)
    W1 = np.asarray(W1, dtype=np.float32)
    b1 = np.asarray(b1, dtype=np.float32)
    W2 = np.asarray(W2, dtype=np.float32)
    b2 = np.asarray(b2, dtype=np.float32)

    order, gates = _route(x, Wg)

    # Dispatch: flatten (token, k) pairs, bucket by expert (stable => slot
    # order within an expert follows token order). Pair p belongs to token p//2.
    expert_flat = order.reshape(-1)  # [2B]
    gate_flat = gates.reshape(-1)  # [2B]
    perm = np.argsort(expert_flat, kind="stable")  # pairs grouped by expert
    counts = np.bincount(expert_flat, minlength=E)
    offs = np.concatenate(([0], np.cumsum(counts)))[:E]

    S1, S2, pieces = _plan_segments(counts)
    C = S1 + S2
    assert C <= 4864, f"unexpectedly imbalanced routing: {counts}"

    # Per-pair placement (core, column) for the combine step, and per-core
    # token lists for the dispatch.
    core_of_pair = np.empty(2 * B, dtype=np.int64)
    col_of_pair = np.empty(2 * B, dtype=np.int64)
    xT = np.ascontiguousarray(x.T)  # [D, B]
    bf16_w1 = [None] * E
    bf16_w2 = [None] * E
    in_maps = []
    for core in range(E):
        xg = np.zeros((D, C), dtype=_BF16)
        in_map = {"xt": xg}
        for seg, (e, tok_start, ln) in enumerate(pieces[core]):
            seg_off = 0 if seg == 0 else S1
            if ln:
                pair_idx = perm[offs[e] + tok_start : offs[e] + tok_start + ln]
                toks = pair_idx // 2
                xg[:, seg_off : seg_off + ln] = xT[:, toks].astype(_BF16)
                core_of_pair[pair_idx] = core
                col_of_pair[pair_idx] = seg_off + np.arange(ln)
            if bf16_w1[e] is None:
                bf16_w1[e] = W1[e].astype(_BF16)
                bf16_w2[e] = W2[e].astype(_BF16)
            s = "ab"[seg]
            in_map[f"w1{s}"] = bf16_w1[e]
            in_map[f"w2{s}"] = bf16_w2[e]
            in_map[f"b1{s}r"] = np.ascontiguousarray(b1[e].reshape(MH, P).T)
        in_maps.append(in_map)

    nc = _build_program(S1, S2)

    from concourse.bass_utils import run_bass_kernel_spmd

    trace = os.environ.get("MOE_TRACE") == "1"
    kwargs = {}
    if trace:
        kwargs = dict(trace=True, trace_cores=list(range(E)))
    res = run_bass_kernel_spmd(nc, in_maps, core_ids=list(range(E)), **kwargs)
    global LAST_RESULTS
    LAST_RESULTS = res

    Y = np.stack([r["yt"] for r in res.results])  # [E, D, C] f32

    # Combine: pair p contributes gate_p * (y[:, col_p] + b2[e_p]) to token
    # p//2. Pairs of token b sit at flat positions 2b, 2b+1.
    cols = Y[core_of_pair, :, col_of_pair]  # [2B, D]
    weighted = (cols + b2[expert_flat]) * gate_flat[:, None]
    out = weighted[0::2] + weighted[1::2]
    return np.ascontiguousarray(out, dtype=np.float32)


# revision 35
# speedup vs baseline: 1.0120x; 1.0108x over previous
"""MoE layer (B=8192, D=1024, E=8, top-2, H=2048) on 8 TRN2 NeuronCores.

Strategy (expert-parallel with two-segment load balancing):
  - Host: gate logits = x @ Wg (fp32), exact top-2 (jax tie-break semantics),
    softmax over the 2 picks. The 16384 (token, expert) pairs are grouped by
    expert and each expert's list is cut into pieces assigned to 16 fixed-size
    segments (8 of size S1, 8 of size S2, one of each per core) so every core
    carries ~the mean load instead of the max expert's load.
  - Device (SPMD): each core runs two segments; segment rows use that
    segment's expert weights: y = relu(x @ W1[e] + b1[e]) @ W2[e] in bf16
    with fp32 PSUM accumulation. Weights resident in SBUF, tokens processed
    in column chunks of <=512.
  - Host: weighted combine out[tok] = sum_k gate * (y + b2[e]).

The expert MLPs are the only O(B*D*H) work; gating/combine are O(B*D).
"""

import os

import numpy as np
import ml_dtypes

B, D, E, TOP_K = 8192, 1024, 8, 2
H = 2 * D
P = 128
CHUNK = 512

KD = D // P  # 8 contraction tiles for mm1 (over D)
MH = H // P  # 16 output tiles for mm1 / contraction tiles for mm2 (over H)
MD = D // P  # 8 output tiles for mm2 (over D)

_BF16 = np.dtype(ml_dtypes.bfloat16)

LAST_RESULTS = None  # BassKernelResults of the most recent run (for test harness)


def _chunk_sizes(n):
    """Split n columns into matmul chunks <=512, avoiding tiny tails."""
    chunks = [CHUNK] * (n // CHUNK)
    tail = n % CHUNK
    if tail >= 128 or not chunks:
        if tail:
            chunks.append(tail)
    elif tail:
        last = chunks.pop() + tail
        chunks += [last - last // 2, last // 2]
    return chunks


def _plan_segments(counts):
    """Pick segment sizes (S1 >= S2) and cut experts into 8 S1-pieces and
    8 S2-pieces (one of each per core). Returns (S1, S2, pieces) where
    pieces[core] = [(expert, tok_start, fill_len), (expert, tok_start, fill_len)]
    for the S1 and S2 segment respectively."""
    order = np.argsort(-counts, kind="stable")
    best = None
    for k in range(0, E // 2 + 1):
        big = order[:k]
        small = order[E - k :] if k else order[:0]
        mid = order[k : E - k]
        S1 = max((int(-(-counts[e] // 2)) for e in big), default=0)
        S2 = max((int(-(-counts[e] // 2)) for e in small), default=0)
        if len(mid):
            if S1 == 0:  # k == 0: all experts are (S1, S2)
                S1 = int(-(-counts[mid].max() // 2))
            S2 = max(S2, int(counts[mid].max()) - S1)
        C = S1 + S2
        if best is None or C < best[0]:
            best = (C, k, S1, S2)
    _, k, S1, S2 = best
    S1 = int(-(-S1 // 8) * 8)  # align for DMA friendliness
    S2 = int(-(-S2 // 8) * 8)

    big = order[:k]
    small = order[E - k :] if k else order[:0]
    mid = order[k : E - k]
    s1_pieces = []  # (expert, tok_start, len)
    s2_pieces = []
    for e in big:
        n = int(counts[e])
        h1 = -(-n // 2)
        s1_pieces += [(int(e), 0, h1), (int(e), h1, n - h1)]
    for e in small:
        n = int(counts[e])
        h1 = -(-n // 2)
        s2_pieces += [(int(e), 0, h1), (int(e), h1, n - h1)]
    for e in mid:
        n = int(counts[e])
        a = min(n, S1)
        s1_pieces.append((int(e), 0, a))
        s2_pieces.append((int(e), a, n - a))
    assert len(s1_pieces) == E and len(s2_pieces) == E
    assert all(ln <= S1 for _, _, ln in s1_pieces)
    assert all(ln <= S2 for _, _, ln in s2_pieces)
    pieces = [[s1_pieces[i], s2_pieces[i]] for i in range(E)]
    return S1, S2, pieces


def _build_program(S1, S2):
    import concourse.bacc as bacc
    import concourse.mybir as mybir
    import concourse.tile as tile
    from concourse.bass import ts

    C = S1 + S2
    nc = bacc.Bacc("TRN2", target_bir_lowering=False, debug=False)
    bf16 = mybir.dt.bfloat16
    f32 = mybir.dt.float32

    xt_d = nc.dram_tensor("xt", (D, C), bf16, kind="ExternalInput").ap()
    w1_ds, w2_ds, b1_ds = [], [], []
    for s in ("a", "b"):
        w1_ds.append(nc.dram_tensor(f"w1{s}", (D, H), bf16, kind="ExternalInput").ap())
        w2_ds.append(nc.dram_tensor(f"w2{s}", (H, D), bf16, kind="ExternalInput").ap())
        b1_ds.append(
            nc.dram_tensor(f"b1{s}r", (P, MH), f32, kind="ExternalInput").ap()
        )
    yt_d = nc.dram_tensor("yt", (D, C), f32, kind="ExternalOutput").ap()

    with tile.TileContext(nc) as tc:
        with (
            tc.tile_pool(name="weights", bufs=1) as wpool,
            tc.tile_pool(name="xin", bufs=1) as xpool,
            tc.tile_pool(name="hbuf", bufs=1) as hpool,
            tc.tile_pool(name="ystage", bufs=3) as ypool,
            tc.tile_pool(name="ps", bufs=8, space="PSUM") as pspool,
        ):
            xt_sb = xpool.tile([P, KD, C], bf16)
            xt_r = xt_d.rearrange("(ko p) c -> p ko c", p=P)
            w1_sbs, w2_sbs, b1_sbs = [], [], []
            w1_rs, w2_rs = [], []
            for s in range(2):
                w1_sbs.append(wpool.tile([P, KD, H], bf16, name=f"w1sb{s}"))
                w2_sbs.append(wpool.tile([P, MH, D], bf16, name=f"w2sb{s}"))
                b1_sbs.append(wpool.tile([P, MH], f32, name=f"b1sb{s}"))
                w1_rs.append(w1_ds[s].rearrange("(ko p) h -> p ko h", p=P))
                w2_rs.append(w2_ds[s].rearrange("(ko p) d -> p ko d", p=P))

            # DMA issue order == need order, alternating pieces of each class
            # between sync (8 HWDGE queues) and gpsimd (8 SWDGE queues) so the
            # early phase gets ~16-queue aggregate bandwidth. The first ~40us
            # of compute consumes w1a+xt(c0)+w2a = 9MB (~250GB/s demand),
            # above the 8-queue ceiling.
            def dma(i, dst, src):
                nc.sync.dma_start(dst, src)

            for i, k in enumerate(range(0, KD, 2)):  # w1a m-group 0: 4 x 64KB
                dma(i, w1_sbs[0][:, k : k + 2, 0:128], w1_rs[0][:, k : k + 2, 0:128])
            for k in range(KD):  # xt chunk 0: 8 x 128KB
                dma(k, xt_sb[:, k, 0:CHUNK], xt_r[:, k, 0:CHUNK])
            for i, k in enumerate(range(0, KD, 2)):  # w1a m-groups 1..3
                dma(
                    i,
                    w1_sbs[0][:, k : k + 2, 128:512],
                    w1_rs[0][:, k : k + 2, 128:512],
                )
            for k in range(KD):  # w1a m-groups 4..15: 8 x 384KB
                dma(k, w1_sbs[0][:, k, 512:H], w1_rs[0][:, k, 512:H])
            dma(0, b1_sbs[0], b1_ds[0])
            dma(1, b1_sbs[1], b1_ds[1])
            for i, k in enumerate(range(0, MH, 2)):  # w2a: 8 x 512KB (~40us in)
                dma(i, w2_sbs[0][:, k : k + 2], w2_rs[0][:, k : k + 2])
            for k in range(KD):  # xt remaining cols (chunk 1 needed ~65us in)
                dma(k, xt_sb[:, k, CHUNK:C], xt_r[:, k, CHUNK:C])
            for k in range(KD):  # segment B weights: needed ~120us in
                dma(k, w1_sbs[1][:, k], w1_rs[1][:, k])
            for i, k in enumerate(range(0, MH, 2)):
                dma(i, w2_sbs[1][:, k : k + 2], w2_rs[1][:, k : k + 2])

            # Chunk schedule: mm2 lags mm1 by one chunk. This keeps w2a off
            # the first ~65us of the DMA critical path and the PE never waits
            # on the ACT h-tiles of the chunk it just produced.
            chunk_list = []  # (seg, off, tw)
            off = 0
            for seg, seg_len in ((0, S1), (1, S2)):
                for tw in _chunk_sizes(seg_len):
                    chunk_list.append((seg, off, tw))
                    off += tw
            # Keep chunk 0 first (the DMA critical path is tuned for it) but
            # end on the smallest chunk so the final copy+DMA trail is short.
            first = chunk_list[0]
            rest = chunk_list[1:]
            tail_chunk = min(rest, key=lambda c: c[2])
            rest.remove(tail_chunk)
            chunk_list = [first] + rest + [tail_chunk]

            def mm2_phase(seg, off, tw, h_sb, k2_outer):
                # k2-outer: all 8 output banks accumulate together so each
                # w2[k2] slice is consumed as it lands (spreads the 4MB w2
                # demand over ~27us instead of needing it all upfront). The
                # last chunk uses m2-outer instead so its copies/output DMAs
                # overlap its own matmul stream rather than trailing it.
                w2_sb = w2_sbs[seg]
                if k2_outer:
                    pys = [
                        pspool.tile([P, CHUNK], f32, tag="ps", name=f"py{m2}")
                        for m2 in range(MD)
                    ]
                    for k2 in range(MH):
                        for m2 in range(MD):
                            nc.tensor.matmul(
                                pys[m2][:, :tw],
                                w2_sb[:, k2, ts(m2, P)],
                                h_sb[:, k2, :tw],
                                start=(k2 == 0),
                                stop=(k2 == MH - 1),
                            )
                    for m2 in range(MD):
                        y_sb = ypool.tile([P, CHUNK], f32, tag="y")
                        nc.vector.tensor_copy(y_sb[:, :tw], pys[m2][:, :tw])
                        nc.sync.dma_start(
                            yt_d[ts(m2, P), off : off + tw], y_sb[:, :tw]
                        )
                else:
                    for m2 in range(MD):
                        py = pspool.tile([P, CHUNK], f32, tag="ps", name="py")
                        for k2 in range(MH):
                            nc.tensor.matmul(
                                py[:, :tw],
                                w2_sb[:, k2, ts(m2, P)],
                                h_sb[:, k2, :tw],
                                start=(k2 == 0),
                                stop=(k2 == MH - 1),
                            )
                        y_sb = ypool.tile([P, CHUNK], f32, tag="y")
                        nc.vector.tensor_copy(y_sb[:, :tw], py[:, :tw])
                        nc.sync.dma_start(
                            yt_d[ts(m2, P), off : off + tw], y_sb[:, :tw]
                        )

            # PE warmup: ~45 junk matmuls on a memset tile run while the first
            # weight/activation DMAs land, so the HAM clock gate is already at
            # 8/8 when real matmuls start.
            warm_sb = xpool.tile([P, P], bf16, name="warm")
            nc.vector.memset(warm_sb, 0.0)
            warm_ps = pspool.tile([P, P], f32, tag="ps", name="warm_ps")
            for _ in range(45):
                nc.tensor.matmul(warm_ps, warm_sb, warm_sb, start=True, stop=True)

            for ci, (seg, off, tw) in enumerate(chunk_list):
                w1_sb, b1_sb = w1_sbs[seg], b1_sbs[seg]
                h_sb = hpool.tile([P, MH, CHUNK], bf16, tag="h")
                for m in range(MH):
                    ph = pspool.tile([P, CHUNK], f32, tag="ps", name="ph")
                    for k in range(KD):
                        nc.tensor.matmul(
                            ph[:, :tw],
                            w1_sb[:, k, ts(m, P)],
                            xt_sb[:, k, off : off + tw],
                            start=(k == 0),
                            stop=(k == KD - 1),
                        )
                    nc.scalar.activation(
                        h_sb[:, m, :tw],
                        ph[:, :tw],
                        mybir.ActivationFunctionType.Relu,
                        bias=b1_sb[:, m : m + 1],
                    )
                    if ci == 0 and m < 10:
                        # Fill DMA-ramp bubbles with dependency-free matmuls
                        # so the HAM clock gate stays at 8/8 while chunk-0
                        # weights stream in.
                        for _ in range(6):
                            nc.tensor.matmul(
                                warm_ps, warm_sb, warm_sb, start=True, stop=True
                            )
                mm2_phase(seg, off, tw, h_sb, k2_outer=(ci < len(chunk_list) - 1))
    nc.finalize()
    return nc


def _route(x, Wg):
    """Exact reference gating on host: top-2 of clean fp32 logits (jax
    tie-break: lower index first), softmax over the two picks."""
    logits = x @ Wg  # [B, E] fp32
    order = np.argsort(-logits, axis=1, kind="stable")[:, :TOP_K]  # [B, 2]
    top_vals = np.take_along_axis(logits, order, axis=1)
    ex = np.exp(top_vals - top_vals[:, :1])  # top_vals sorted desc -> max first
    gates = (ex / ex.sum(axis=1, keepdims=True)).astype(np.float32)  # [B, 2]
    return order, gates


def kernel(x, Wg, W1, b1, W2, b2):
    x = np.ascontiguousarray(np.asarray(x, dtype=np.float32))
    Wg = np.asarray(Wg, dtype=np.float32)
    W1 = np.asarray(W1, dtype=np.float32)
    b1 = np.asarray(b1, dtype=np.float32)
    W2 = np.asarray(W2, dtype=np.float32)
    b2 = np.asarray(b2, dtype=np.float32)

    order, gates = _route(x, Wg)

    # Dispatch: flatten (token, k) pairs, bucket by expert (stable => slot
    # order within an expert follows token order). Pair p belongs to token p//2.
    expert_flat = order.reshape(-1)  # [2B]
    gate_flat = gates.reshape(-1)  # [2B]
    perm = np.argsort(expert_flat, kind="stable")  # pairs grouped by expert
    counts = np.bincount(expert_flat, minlength=E)
    offs = np.concatenate(([0], np.cumsum(counts)))[:E]

    S1, S2, pieces = _plan_segments(counts)
    C = S1 + S2
    assert C <= 4864, f"unexpectedly imbalanced routing: {counts}"

    # Per-pair placement (core, column) for the combine step, and per-core
    # token lists for the dispatch.
    core_of_pair = np.empty(2 * B, dtype=np.int64)
    col_of_pair = np.empty(2 * B, dtype=np.int64)
    xT = np.ascontiguousarray(x.T)  # [D, B]
    bf16_w1 = [None] * E
    bf16_w2 = [None] * E
    in_maps = []
    for core in range(E):
        xg = np.zeros((D, C), dtype=_BF16)
        in_map = {"xt": xg}
        for seg, (e, tok_start, ln) in enumerate(pieces[core]):
            seg_off = 0 if seg == 0 else S1
            if ln:
                pair_idx = perm[offs[e] + tok_start : offs[e] + tok_start + ln]
                toks = pair_idx // 2
                xg[:, seg_off : seg_off + ln] = xT[:, toks].astype(_BF16)
                core_of_pair[pair_idx] = core
                col_of_pair[pair_idx] = seg_off + np.arange(ln)
            if bf16_w1[e] is None:
                bf16_w1[e] = W1[e].astype(_BF16)
                bf16_w2[e] = W2[e].astype(_BF16)
            s = "ab"[seg]
            in_map[f"w1{s}"] = bf16_w1[e]
            in_map[f"w2{s}"] = bf16_w2[e]
            in_map[f"b1{s}r"] = np.ascontiguousarray(b1[e].reshape(MH, P).T)
        in_maps.append(in_map)

    nc = _build_program(S1, S2)

    from concourse.bass_utils import run_bass_kernel_spmd

    trace = os.environ.get("MOE_TRACE") == "1"
    kwargs = {}
    if trace:
        kwargs = dict(trace=True, trace_cores=list(range(E)))
    try:
        res = run_bass_kernel_spmd(nc, in_maps, core_ids=list(range(E)), **kwargs)
    except Exception:  # wedged accelerator: reset once and retry untraced
        try:
            import ctypes

            lib = ctypes.CDLL("/opt/axon/libaxon_pjrt.so")
            lib.axon_reset.restype = ctypes.c_int64
            lib.axon_reset()
        except OSError:
            pass
        res = run_bass_kernel_spmd(nc, in_maps, core_ids=list(range(E)))
    global LAST_RESULTS
    LAST_RESULTS = res

    Y = np.stack([r["yt"] for r in res.results])  # [E, D, C] f32

    # Combine: pair p contributes gate_p * (y[:, col_p] + b2[e_p]) to token
    # p//2. Pairs of token b sit at flat positions 2b, 2b+1.
    cols = Y[core_of_pair, :, col_of_pair]  # [2B, D]
    weighted = (cols + b2[expert_flat]) * gate_flat[:, None]
    out = weighted[0::2] + weighted[1::2]
    return np.ascontiguousarray(out, dtype=np.float32)
